# revision 6
# baseline (speedup 1.0000x reference)
"""Swin shifted-window attention (nn_AttentionSwinInd) on 8 TRN2 cores.

The wall-clock cost of this op is dominated by moving data over the
axon tunnel (~100MB/s per direction) plus fixed dispatch/launch latency,
not by device compute. Hence:
- Host does a single fused cast(f32->bf16)+cyclic-roll pass; all window
  partitioning / transposition happens on-device.
- Device gathers windows from the natural [wt, h, w, d] layout via
  strided DMAs, PE-transposes each window to feature-major [128, 196],
  runs the windowed MHA, PE-transposes back, quantizes each output token
  to int8 with a per-token f32 scale bit-packed into bytes 128:132 of the
  row (halves the device->host transfer; rel err stays ~8e-3).
- Runner: the bass_exec custom call is jitted ONCE per process (no
  per-call retrace/recompile), outputs are NOT donated zero buffers (the
  kernel writes every element), weights live on-device keyed by a content
  hash, and output shards are fetched/dequantized by a thread pool.
Sharding: core c = (n = c//4, tblk = c%4) owns the 64 windows of one
(batch, temporal-block) pair; weights replicated.
"""

import numpy as np
import ml_dtypes

BF16 = ml_dtypes.bfloat16

N, T, S, D = 2, 16, 3136, 128
H = W = 56
WT, WH, WW = 4, 7, 7
NH, HD = 8, 16
L = WT * WH * WW          # 196
NWIN = 512
NCORES = 8
WPC = NWIN // NCORES      # 64 windows per core (8x8 over hb, wb)
KT0, KT1 = 128, 68        # key tiles (128 + 68 = 196)

IN_NAMES = ["xin", "wq_a", "wq_b", "wk_a", "wk_b", "wv",
            "pw_a", "pw_b", "pb", "ident"]
OUT_NAMES = ["yout"]
DQ = D + 4                # int8 row: 128 data bytes + 4 scale bytes (f32)

_cache = {}


def _build_program():
    import concourse.bass as bass
    import concourse.tile as tile
    from concourse import mybir

    f32 = mybir.dt.float32
    bf16 = mybir.dt.bfloat16
    i8 = mybir.dt.int8

    nc = bass.Bass()

    xin = nc.declare_dram_parameter("xin", [WT, H, W, D], bf16, isOutput=False)
    wq_a = nc.declare_dram_parameter("wq_a", [128, 128], bf16, isOutput=False)
    wq_b = nc.declare_dram_parameter("wq_b", [128, 128], bf16, isOutput=False)
    wk_a = nc.declare_dram_parameter("wk_a", [128, 128], bf16, isOutput=False)
    wk_b = nc.declare_dram_parameter("wk_b", [128, 128], bf16, isOutput=False)
    wv = nc.declare_dram_parameter("wv", [128, 128], bf16, isOutput=False)
    pw_a = nc.declare_dram_parameter("pw_a", [128, 128], bf16, isOutput=False)
    pw_b = nc.declare_dram_parameter("pw_b", [128, 128], bf16, isOutput=False)
    pb = nc.declare_dram_parameter("pb", [128, 1], f32, isOutput=False)
    ident = nc.declare_dram_parameter("ident", [128, 128], bf16, isOutput=False)
    yout = nc.declare_dram_parameter("yout", [WT, H, W, DQ], i8, isOutput=True)

    EXP = mybir.ActivationFunctionType.Exp

    with tile.TileContext(nc) as tc:
        with (
            tc.tile_pool(name="consts", bufs=1) as consts,
            tc.tile_pool(name="sb", bufs=2) as sb,
            tc.tile_pool(name="esb", bufs=2) as esb,
            tc.tile_pool(name="pbank", bufs=4, space="PSUM") as pbank,
            tc.tile_pool(name="pst", bufs=1, space="PSUM") as pst,
        ):
            # constants
            wq_a_s = consts.tile([128, 128], bf16, tag="wq_a")
            wq_b_s = consts.tile([128, 128], bf16, tag="wq_b")
            wk_a_s = consts.tile([128, 128], bf16, tag="wk_a")
            wk_b_s = consts.tile([128, 128], bf16, tag="wk_b")
            wv_s = consts.tile([128, 128], bf16, tag="wv")
            pw_a_s = consts.tile([128, 128], bf16, tag="pw_a")
            pw_b_s = consts.tile([128, 128], bf16, tag="pw_b")
            pb_s = consts.tile([128, 1], f32, tag="pb")
            id_s = consts.tile([128, 128], bf16, tag="ident")
            ones_s = consts.tile([128, 17], bf16, tag="ones")
            nc.sync.dma_start(out=wq_a_s, in_=wq_a[:, :])
            nc.sync.dma_start(out=wq_b_s, in_=wq_b[:, :])
            nc.sync.dma_start(out=wk_a_s, in_=wk_a[:, :])
            nc.sync.dma_start(out=wk_b_s, in_=wk_b[:, :])
            nc.sync.dma_start(out=wv_s, in_=wv[:, :])
            nc.sync.dma_start(out=pw_a_s, in_=pw_a[:, :])
            nc.sync.dma_start(out=pw_b_s, in_=pw_b[:, :])
            nc.sync.dma_start(out=pb_s, in_=pb[:, :])
            nc.sync.dma_start(out=id_s, in_=ident[:, :])
            nc.vector.memset(ones_s, 1.0)

            for w in range(WPC):
                hb, wb = w // 8, w % 8
                hs, ws = 7 * hb, 7 * wb

                # --- load natural layout: two tiles of 2 wt-planes each
                # (DMA APs are limited to 3 dims, so one DMA per wt-plane)
                xn0 = sb.tile([98, 128], bf16, tag="xn0")
                xn1 = sb.tile([98, 128], bf16, tag="xn1")
                for wt, (dst, r0) in enumerate(
                        ((xn0, 0), (xn0, 49), (xn1, 0), (xn1, 49))):
                    nc.sync.dma_start(
                        out=dst[r0:r0 + 49, :],
                        in_=xin[wt, hs:hs + 7, ws:ws + 7, :])

                # --- PE transpose to feature-major x^T [128, 196]
                xt0_p = pbank.tile([128, 98], bf16, tag="pb")
                xt1_p = pbank.tile([128, 98], bf16, tag="pb")
                nc.tensor.transpose(xt0_p, xn0, id_s[0:98, 0:98])
                nc.tensor.transpose(xt1_p, xn1, id_s[0:98, 0:98])
                xt = sb.tile([128, L], bf16, tag="xt")
                nc.vector.tensor_copy(xt[:, 0:98], xt0_p)
                nc.vector.tensor_copy(xt[:, 98:196], xt1_p)

                # --- Q^T, K^T (A/B halves, head h at partitions 32h..32h+15)
                qa_p = pbank.tile([128, L], f32, tag="pb")
                qb_p = pbank.tile([128, L], f32, tag="pb")
                ka_p = pbank.tile([128, L], f32, tag="pb")
                kb_p = pbank.tile([128, L], f32, tag="pb")
                nc.tensor.matmul(qa_p, wq_a_s, xt, start=True, stop=True)
                nc.tensor.matmul(qb_p, wq_b_s, xt, start=True, stop=True)
                nc.tensor.matmul(ka_p, wk_a_s, xt, start=True, stop=True)
                nc.tensor.matmul(kb_p, wk_b_s, xt, start=True, stop=True)
                qa = sb.tile([128, L], bf16, tag="qa")
                qb = sb.tile([128, L], bf16, tag="qb")
                ka = sb.tile([128, L], bf16, tag="ka")
                kb = sb.tile([128, L], bf16, tag="kb")
                nc.vector.tensor_copy(qa, qa_p)
                nc.vector.tensor_copy(qb, qb_p)
                nc.vector.tensor_copy(ka, ka_p)
                nc.vector.tensor_copy(kb, kb_p)

                # --- V natural [tokens, 128], two key tiles, with ones col
                vp0 = pbank.tile([128, 128], f32, tag="pb")
                vp1 = pbank.tile([KT1, 128], f32, tag="pb")
                nc.tensor.matmul(vp0, xt[:, 0:KT0], wv_s, start=True, stop=True)
                nc.tensor.matmul(vp1, xt[:, KT0:L], wv_s, start=True, stop=True)
                va0 = sb.tile([128, 8, 17], bf16, tag="va0")
                va1 = sb.tile([128, 8, 17], bf16, tag="va1")
                nc.vector.memset(va0[:, :, 0:1], 1.0)
                nc.vector.memset(va1[0:KT1, :, 0:1], 1.0)
                nc.vector.tensor_copy(
                    va0[:, :, 1:17], vp0.rearrange("p (h d) -> p h d", h=8))
                nc.vector.tensor_copy(
                    va1[0:KT1, :, 1:17], vp1.rearrange("p (h d) -> p h d", h=8))

                yt_p = pbank.tile([128, L], f32, tag="pb")

                for half, (qh, kh, hoff) in enumerate(
                        ((qa, ka, 0), (qb, kb, 4))):
                    # --- scores: ST[key, query] per head, 4x row-tiled
                    st = pst.tile([128, 4, 512], f32, tag="st")
                    for h in range(4):
                        p0 = 32 * h
                        nc.tensor.matmul(
                            st[:, h, 0:L],
                            kh[p0:p0 + 16, 0:KT0],
                            qh[p0:p0 + 16, :],
                            start=True, stop=True, tile_position=(p0, 0))
                        nc.tensor.matmul(
                            st[0:KT1, h, L:2 * L],
                            kh[p0:p0 + 16, KT0:L],
                            qh[p0:p0 + 16, :],
                            start=True, stop=True, tile_position=(p0, 0))
                    e = esb.tile([128, 4, 2 * L], bf16, tag="e")
                    nc.scalar.activation(e, st[:, :, 0:2 * L], EXP)

                    # --- PV with ones column: row 32h = denom, +1..+16 = O^T
                    ot_p = pbank.tile([128, L], f32, tag="pb")
                    for h in range(4):
                        p0 = 32 * h
                        nc.tensor.matmul(
                            ot_p[p0:p0 + 17, :],
                            va0[:, hoff + h, :],
                            e[0:KT0, h, 0:L],
                            start=True, stop=False, tile_position=(0, p0))
                        nc.tensor.matmul(
                            ot_p[p0:p0 + 17, :],
                            va1[0:KT1, hoff + h, :],
                            e[0:KT1, h, L:2 * L],
                            start=False, stop=True, tile_position=(0, p0))

                    # --- normalize: recip, K=1 broadcast matmul, multiply
                    rec = sb.tile([128, L], bf16, tag="rec")
                    with nc.allow_low_precision(reason="softmax denom recip"):
                        nc.vector.reciprocal(rec, ot_p)
                    b_p = pbank.tile([128, L], f32, tag="pb")
                    for h in range(4):
                        p0 = 32 * h
                        nc.tensor.matmul(
                            b_p[p0:p0 + 17, :],
                            ones_s[p0:p0 + 1, :],
                            rec[p0:p0 + 1, :],
                            start=True, stop=True, tile_position=(p0, p0))
                    bsb = sb.tile([128, L], bf16, tag="bsb")
                    nc.scalar.copy(bsb, b_p)
                    onrm = sb.tile([128, L], bf16, tag="onrm")
                    nc.vector.tensor_mul(onrm, ot_p, bsb)

                    # --- projection accumulate
                    pw_s = pw_a_s if half == 0 else pw_b_s
                    nc.tensor.matmul(yt_p, pw_s, onrm,
                                     start=(half == 0), stop=(half == 1))

                # --- bias add (bf16 out), PE transpose back, scatter out
                yt16 = sb.tile([128, L], bf16, tag="yt16")
                with nc.allow_low_precision(reason="bf16 output"):
                    nc.vector.tensor_scalar_add(yt16, yt_p, pb_s)
                yn0_p = pbank.tile([98, 128], bf16, tag="pb")
                yn1_p = pbank.tile([98, 128], bf16, tag="pb")
                nc.tensor.transpose(yn0_p, yt16[:, 0:98], id_s)
                nc.tensor.transpose(yn1_p, yt16[:, 98:196], id_s)
                # --- int8 per-token quantization: q = y * (127/absmax),
                # f32 scale bit-packed into bytes 128:132 of each row
                for wt0, src in ((0, yn0_p), (2, yn1_p)):
                    m = sb.tile([98, 1], f32, tag="qm")
                    nc.vector.tensor_reduce(
                        m, src, mybir.AxisListType.X, mybir.AluOpType.max,
                        apply_absolute_value=True)
                    sc = sb.tile([98, 1], f32, tag="qsc")   # absmax/127
                    nc.vector.tensor_scalar(
                        sc, m, 1.0 / 127.0, 1e-30,
                        op0=mybir.AluOpType.mult, op1=mybir.AluOpType.max)
                    r = sb.tile([98, 1], f32, tag="qr")
                    nc.vector.reciprocal(r, sc)
                    q = sb.tile([98, DQ], i8, tag="q")
                    with nc.allow_low_precision(reason="int8 quantize"):
                        nc.vector.tensor_scalar_mul(q[:, 0:D], src, r)
                    nc.vector.tensor_copy(
                        q[:, D:DQ], sc.bitcast(i8))
                    for k in range(2):
                        nc.sync.dma_start(
                            out=yout[wt0 + k, hs:hs + 7, ws:ws + 7, :],
                            in_=q[49 * k:49 * k + 49, :])

    _split_mm_waits(nc, mybir)
    return nc


def _split_mm_waits(nc, mybir):
    """Walrus allows only one sync-wait on a Matmult: move extra waits onto
    PE NoOps inserted just before the matmul (same engine stream, absolute
    sem-ge waits, so waiting earlier is equivalent)."""
    for fn in nc.m.functions:
        for bb in fn.blocks:
            il = bb.instructions
            i = 0
            while i < len(il):
                inst = il[i]
                si = getattr(inst, "sync_info", None)
                if (not isinstance(inst, mybir.InstNoOp) and si is not None
                        and si.on_wait and len(si.on_wait) > 1):
                    waits = list(si.on_wait)
                    for wsel in waits[:-1]:
                        nop = mybir.InstNoOp(
                            name=nc.get_next_instruction_name(),
                            sync_info=mybir.SyncInfo(
                                on_wait=[wsel], on_update=[]),
                            bass_nofuse=True,
                            engine=inst.engine,
                        )
                        il.insert(i, nop)
                        i += 1
                    inst.sync_info = mybir.SyncInfo(
                        on_wait=[waits[-1]], on_update=list(si.on_update))
                i += 1


def _get_runner():
    if "runner" in _cache:
        return _cache["runner"]

    import jax
    from jax.sharding import Mesh, PartitionSpec
    from jax.experimental.shard_map import shard_map
    from concourse import mybir
    from concourse.bass2jax import (
        _bass_exec_p, install_neuronx_cc_hook, partition_id_tensor)

    install_neuronx_cc_hook()
    nc = _build_program()

    partition_name = (nc.partition_id_tensor.name
                      if nc.partition_id_tensor else None)
    in_names, out_names, out_avals = [], [], []
    for alloc in nc.m.functions[0].allocations:
        if not isinstance(alloc, mybir.MemoryLocationSet):
            continue
        name = alloc.memorylocations[0].name
        if alloc.kind == "ExternalInput":
            if name != partition_name:
                in_names.append(name)
        elif alloc.kind == "ExternalOutput":
            out_names.append(name)
            out_avals.append(jax.core.ShapedArray(
                tuple(alloc.tensor_shape), mybir.dt.np(alloc.dtype)))
    assert in_names == IN_NAMES, in_names
    assert out_names == OUT_NAMES, out_names
    in_names_all = list(in_names)
    if partition_name is not None:
        in_names_all.append(partition_name)

    def _body(*args):
        operands = list(args)
        if partition_name is not None:
            operands.append(partition_id_tensor())
        outs = _bass_exec_p.bind(
            *operands,
            out_avals=tuple(out_avals),
            in_names=tuple(in_names_all),
            out_names=tuple(out_names),
            lowering_input_output_aliases=(),
            sim_require_finite=True,
            sim_require_nnan=True,
            nc=nc,
        )
        return tuple(outs)

    devices = jax.devices()[:NCORES]
    mesh = Mesh(np.asarray(devices), ("core",))
    sharded = jax.jit(
        shard_map(
            _body, mesh=mesh,
            in_specs=(PartitionSpec("core"),) * len(in_names),
            out_specs=(PartitionSpec("core"),) * len(out_names),
            check_rep=False,
        ),
        keep_unused=True,
    )
    _cache["mesh"] = mesh
    _cache["runner"] = sharded
    return sharded


def _weights_glob(qkv_w, proj_w, proj_b):
    """Device-resident replicated weight arrays, re-uploaded only when the
    weight bytes change."""
    import hashlib
    import jax
    from jax.sharding import NamedSharding, PartitionSpec

    dig = hashlib.blake2b(qkv_w.tobytes(), digest_size=16)
    dig.update(proj_w.tobytes())
    dig.update(proj_b.tobytes())
    key = dig.hexdigest()
    ent = _cache.get("wglob")
    if ent is not None and ent[0] == key:
        return ent[1]
    wd = _prep_weights(qkv_w, proj_w, proj_b)
    sh = NamedSharding(_cache["mesh"], PartitionSpec("core"))
    dws = [jax.device_put(np.concatenate([wd[n]] * NCORES, axis=0), sh)
           for n in IN_NAMES[1:]]
    jax.block_until_ready(dws)
    _cache["wglob"] = (key, dws)
    return dws


def _prep_weights(qkv_w, proj_w, proj_b):
    Wq = qkv_w[0:128] * (HD ** -0.5)
    Wk = qkv_w[128:256]
    Wv = qkv_w[256:384]

    def head_pad_T(Wm):
        # out[di, 32h+j] = Wm[16h+j, di] for 4 heads, rest zero
        out_a = np.zeros((128, 128), np.float32)
        out_b = np.zeros((128, 128), np.float32)
        for h in range(4):
            out_a[:, 32 * h:32 * h + 16] = Wm[16 * h:16 * h + 16].T
            out_b[:, 32 * h:32 * h + 16] = Wm[16 * (h + 4):16 * (h + 4) + 16].T
        return out_a.astype(BF16), out_b.astype(BF16)

    wq_a, wq_b = head_pad_T(Wq)
    wk_a, wk_b = head_pad_T(Wk)
    wv = Wv.T.astype(BF16)

    # proj lhsT: row 32h+1+j of O^T layout corresponds to di = 16h+j
    pw_a = np.zeros((128, 128), np.float32)
    pw_b = np.zeros((128, 128), np.float32)
    for h in range(4):
        pw_a[32 * h + 1:32 * h + 17, :] = proj_w[:, 16 * h:16 * h + 16].T
        pw_b[32 * h + 1:32 * h + 17, :] = \
            proj_w[:, 16 * (h + 4):16 * (h + 4) + 16].T
    pw_a = pw_a.astype(BF16)
    pw_b = pw_b.astype(BF16)
    pb = proj_b.reshape(128, 1).astype(np.float32)
    ident = np.eye(128, dtype=np.float32).astype(BF16)
    return dict(wq_a=wq_a, wq_b=wq_b, wk_a=wk_a, wk_b=wk_b, wv=wv,
                pw_a=pw_a, pw_b=pw_b, pb=pb, ident=ident)


# wrap-split slices for the fused cast+roll passes
def _roll_chunks(size, shift):
    """dst[i] = src[(i + shift) % size] as a list of (dst_slice, src_slice)."""
    shift %= size
    if shift == 0:
        return [((0, size), (0, size))]
    return [((0, size - shift), (shift, size)),
            ((size - shift, size), (0, shift))]


def _pool():
    if "pool" not in _cache:
        from concurrent.futures import ThreadPoolExecutor
        _cache["pool"] = ThreadPoolExecutor(max_workers=16)
    return _cache["pool"]


def _prep_x(x):
    """f32 (N,T,S,D) -> bf16 rolled (N,T,H,W,D), one fused pass
    (parallelized over T; numpy casts release the GIL). The staging
    buffer is reused across calls (it is consumed by the device upload
    before the next call starts)."""
    x5 = x.reshape(N, T, H, W, D)
    if "xr" not in _cache:
        _cache["xr"] = np.empty((N, T, H, W, D), BF16)
    xr = _cache["xr"]
    hw = [(dh, sh, dw, sw)
          for dh, sh in _roll_chunks(H, 4)
          for dw, sw in _roll_chunks(W, 4)]

    def do_t(dt):
        st = (dt + 2) % T
        for dh, sh, dw, sw in hw:
            xr[:, dt, dh[0]:dh[1], dw[0]:dw[1]] = \
                x5[:, st, sh[0]:sh[1], sw[0]:sw[1]]
    list(_pool().map(do_t, range(T)))
    return xr


def kernel(x, qkv_w, proj_w, proj_b):
    x = np.asarray(x, np.float32)
    qkv_w = np.asarray(qkv_w, np.float32)
    proj_w = np.asarray(proj_w, np.float32)
    proj_b = np.asarray(proj_b, np.float32)

    runner = _get_runner()

    xr = _prep_x(x)                                # [2,16,56,56,128] bf16
    xg = xr.reshape(NCORES * WT, H, W, D)          # global input, view
    wglob = _weights_glob(qkv_w, proj_w, proj_b)

    (yg,) = runner(xg, *wglob)        # [32,56,56,132] int8 (data + scale)

    shards = list(yg.addressable_shards)
    for s in shards:
        s.data.copy_to_host_async()

    # per-shard: fetch + dequant + reverse roll (T by +2, H/W by +3)
    y5 = np.empty((N, T, H, W, D), np.float32)
    h_chunks = _roll_chunks(H, -3)   # dst[i] = src[(i-3) % 56]

    def do_shard(s):
        c = s.index[0].start // WT
        n_, tblk = c // 4, c % 4
        arr = np.asarray(s.data)                   # [4,56,56,132] int8
        sc = np.ascontiguousarray(arr[:, :, :, D:DQ]).view('<f4')[..., 0]
        # this shard holds rolled T = 4*tblk + wt -> final T = (+2) % 16
        for wt in range(WT):
            tf = (4 * tblk + wt + 2) % T
            for dh, sh in h_chunks:
                for dw, sw in h_chunks:
                    blk = arr[wt, sh[0]:sh[1], sw[0]:sw[1], 0:D]
                    y5[n_, tf, dh[0]:dh[1], dw[0]:dw[1]] = \
                        blk * sc[wt, sh[0]:sh[1], sw[0]:sw[1], None]
    list(_pool().map(do_shard, shards))
    return y5.reshape(N, T, S, D)


# revision 8
# speedup vs baseline: 1.1167x; 1.1167x over previous
"""Swin shifted-window attention (nn_AttentionSwinInd) on 8 TRN2 cores.

The wall-clock cost of this op is dominated by moving data over the
axon tunnel (~100MB/s per direction) plus fixed dispatch/launch latency,
not by device compute. Hence:
- Host does a single fused cast(f32->bf16)+cyclic-roll pass; all window
  partitioning / transposition happens on-device.
- Device gathers windows from the natural [wt, h, w, d] layout via
  strided DMAs, PE-transposes each window to feature-major [128, 196],
  runs the windowed MHA, PE-transposes back, quantizes each output token
  to int8 with a per-token f32 scale bit-packed into bytes 128:132 of the
  row (halves the device->host transfer; rel err stays ~8e-3).
- Runner: the bass_exec custom call is jitted ONCE per process (no
  per-call retrace/recompile), outputs are NOT donated zero buffers (the
  kernel writes every element), weights live on-device keyed by a content
  hash, and output shards are fetched/dequantized by a thread pool.
Sharding: core c = (n = c//4, tblk = c%4) owns the 64 windows of one
(batch, temporal-block) pair; weights replicated.
"""

import numpy as np
import ml_dtypes

BF16 = ml_dtypes.bfloat16

N, T, S, D = 2, 16, 3136, 128
H = W = 56
WT, WH, WW = 4, 7, 7
NH, HD = 8, 16
L = WT * WH * WW          # 196
NWIN = 512
NCORES = 8
WPC = NWIN // NCORES      # 64 windows per core (8x8 over hb, wb)
KT0, KT1 = 128, 68        # key tiles (128 + 68 = 196)

IN_NAMES = ["xin", "wq_a", "wq_b", "wk_a", "wk_b", "wv",
            "pw_a", "pw_b", "pb", "ident"]
OUT_NAMES = ["yout"]
DQ = D + 4                # int8 row: 128 data bytes + 4 scale bytes (f32)

_cache = {}


def _build_program():
    import concourse.bass as bass
    import concourse.tile as tile
    from concourse import mybir

    f32 = mybir.dt.float32
    bf16 = mybir.dt.bfloat16
    i8 = mybir.dt.int8

    nc = bass.Bass()

    xin = nc.declare_dram_parameter("xin", [WT, H, W, D], bf16, isOutput=False)
    wq_a = nc.declare_dram_parameter("wq_a", [128, 128], bf16, isOutput=False)
    wq_b = nc.declare_dram_parameter("wq_b", [128, 128], bf16, isOutput=False)
    wk_a = nc.declare_dram_parameter("wk_a", [128, 128], bf16, isOutput=False)
    wk_b = nc.declare_dram_parameter("wk_b", [128, 128], bf16, isOutput=False)
    wv = nc.declare_dram_parameter("wv", [128, 128], bf16, isOutput=False)
    pw_a = nc.declare_dram_parameter("pw_a", [128, 128], bf16, isOutput=False)
    pw_b = nc.declare_dram_parameter("pw_b", [128, 128], bf16, isOutput=False)
    pb = nc.declare_dram_parameter("pb", [128, 1], f32, isOutput=False)
    ident = nc.declare_dram_parameter("ident", [128, 128], bf16, isOutput=False)
    yout = nc.declare_dram_parameter("yout", [WT, H, W, DQ], i8, isOutput=True)

    EXP = mybir.ActivationFunctionType.Exp

    with tile.TileContext(nc) as tc:
        with (
            tc.tile_pool(name="consts", bufs=1) as consts,
            tc.tile_pool(name="sb", bufs=2) as sb,
            tc.tile_pool(name="esb", bufs=2) as esb,
            tc.tile_pool(name="pbank", bufs=4, space="PSUM") as pbank,
            tc.tile_pool(name="pst", bufs=1, space="PSUM") as pst,
        ):
            # constants
            wq_a_s = consts.tile([128, 128], bf16, tag="wq_a")
            wq_b_s = consts.tile([128, 128], bf16, tag="wq_b")
            wk_a_s = consts.tile([128, 128], bf16, tag="wk_a")
            wk_b_s = consts.tile([128, 128], bf16, tag="wk_b")
            wv_s = consts.tile([128, 128], bf16, tag="wv")
            pw_a_s = consts.tile([128, 128], bf16, tag="pw_a")
            pw_b_s = consts.tile([128, 128], bf16, tag="pw_b")
            pb_s = consts.tile([128, 1], f32, tag="pb")
            id_s = consts.tile([128, 128], bf16, tag="ident")
            ones_s = consts.tile([128, 17], bf16, tag="ones")
            nc.sync.dma_start(out=wq_a_s, in_=wq_a[:, :])
            nc.sync.dma_start(out=wq_b_s, in_=wq_b[:, :])
            nc.sync.dma_start(out=wk_a_s, in_=wk_a[:, :])
            nc.sync.dma_start(out=wk_b_s, in_=wk_b[:, :])
            nc.sync.dma_start(out=wv_s, in_=wv[:, :])
            nc.sync.dma_start(out=pw_a_s, in_=pw_a[:, :])
            nc.sync.dma_start(out=pw_b_s, in_=pw_b[:, :])
            nc.sync.dma_start(out=pb_s, in_=pb[:, :])
            nc.sync.dma_start(out=id_s, in_=ident[:, :])
            nc.vector.memset(ones_s, 1.0)

            for w in range(WPC):
                hb, wb = w // 8, w % 8
                hs, ws = 7 * hb, 7 * wb

                # --- load natural layout: two tiles of 2 wt-planes each
                # (DMA APs are limited to 3 dims, so one DMA per wt-plane)
                xn0 = sb.tile([98, 128], bf16, tag="xn0")
                xn1 = sb.tile([98, 128], bf16, tag="xn1")
                for wt, (dst, r0) in enumerate(
                        ((xn0, 0), (xn0, 49), (xn1, 0), (xn1, 49))):
                    nc.sync.dma_start(
                        out=dst[r0:r0 + 49, :],
                        in_=xin[wt, hs:hs + 7, ws:ws + 7, :])

                # --- PE transpose to feature-major x^T [128, 196]
                xt0_p = pbank.tile([128, 98], bf16, tag="pb")
                xt1_p = pbank.tile([128, 98], bf16, tag="pb")
                nc.tensor.transpose(xt0_p, xn0, id_s[0:98, 0:98])
                nc.tensor.transpose(xt1_p, xn1, id_s[0:98, 0:98])
                xt = sb.tile([128, L], bf16, tag="xt")
                nc.vector.tensor_copy(xt[:, 0:98], xt0_p)
                nc.vector.tensor_copy(xt[:, 98:196], xt1_p)

                # --- Q^T, K^T (A/B halves, head h at partitions 32h..32h+15)
                qa_p = pbank.tile([128, L], f32, tag="pb")
                qb_p = pbank.tile([128, L], f32, tag="pb")
                ka_p = pbank.tile([128, L], f32, tag="pb")
                kb_p = pbank.tile([128, L], f32, tag="pb")
                nc.tensor.matmul(qa_p, wq_a_s, xt, start=True, stop=True)
                nc.tensor.matmul(qb_p, wq_b_s, xt, start=True, stop=True)
                nc.tensor.matmul(ka_p, wk_a_s, xt, start=True, stop=True)
                nc.tensor.matmul(kb_p, wk_b_s, xt, start=True, stop=True)
                qa = sb.tile([128, L], bf16, tag="qa")
                qb = sb.tile([128, L], bf16, tag="qb")
                ka = sb.tile([128, L], bf16, tag="ka")
                kb = sb.tile([128, L], bf16, tag="kb")
                nc.vector.tensor_copy(qa, qa_p)
                nc.vector.tensor_copy(qb, qb_p)
                nc.vector.tensor_copy(ka, ka_p)
                nc.vector.tensor_copy(kb, kb_p)

                # --- V natural [tokens, 128], two key tiles, with ones col
                vp0 = pbank.tile([128, 128], f32, tag="pb")
                vp1 = pbank.tile([KT1, 128], f32, tag="pb")
                nc.tensor.matmul(vp0, xt[:, 0:KT0], wv_s, start=True, stop=True)
                nc.tensor.matmul(vp1, xt[:, KT0:L], wv_s, start=True, stop=True)
                va0 = sb.tile([128, 8, 17], bf16, tag="va0")
                va1 = sb.tile([128, 8, 17], bf16, tag="va1")
                nc.vector.memset(va0[:, :, 0:1], 1.0)
                nc.vector.memset(va1[0:KT1, :, 0:1], 1.0)
                nc.vector.tensor_copy(
                    va0[:, :, 1:17], vp0.rearrange("p (h d) -> p h d", h=8))
                nc.vector.tensor_copy(
                    va1[0:KT1, :, 1:17], vp1.rearrange("p (h d) -> p h d", h=8))

                yt_p = pbank.tile([128, L], f32, tag="pb")

                for half, (qh, kh, hoff) in enumerate(
                        ((qa, ka, 0), (qb, kb, 4))):
                    # --- scores: ST[key, query] per head, 4x row-tiled
                    st = pst.tile([128, 4, 512], f32, tag="st")
                    for h in range(4):
                        p0 = 32 * h
                        nc.tensor.matmul(
                            st[:, h, 0:L],
                            kh[p0:p0 + 16, 0:KT0],
                            qh[p0:p0 + 16, :],
                            start=True, stop=True, tile_position=(p0, 0))
                        nc.tensor.matmul(
                            st[0:KT1, h, L:2 * L],
                            kh[p0:p0 + 16, KT0:L],
                            qh[p0:p0 + 16, :],
                            start=True, stop=True, tile_position=(p0, 0))
                    e = esb.tile([128, 4, 2 * L], bf16, tag="e")
                    nc.scalar.activation(e, st[:, :, 0:2 * L], EXP)

                    # --- PV with ones column: row 32h = denom, +1..+16 = O^T
                    ot_p = pbank.tile([128, L], f32, tag="pb")
                    for h in range(4):
                        p0 = 32 * h
                        nc.tensor.matmul(
                            ot_p[p0:p0 + 17, :],
                            va0[:, hoff + h, :],
                            e[0:KT0, h, 0:L],
                            start=True, stop=False, tile_position=(0, p0))
                        nc.tensor.matmul(
                            ot_p[p0:p0 + 17, :],
                            va1[0:KT1, hoff + h, :],
                            e[0:KT1, h, L:2 * L],
                            start=False, stop=True, tile_position=(0, p0))

                    # --- normalize: recip, K=1 broadcast matmul, multiply
                    rec = sb.tile([128, L], bf16, tag="rec")
                    with nc.allow_low_precision(reason="softmax denom recip"):
                        nc.vector.reciprocal(rec, ot_p)
                    b_p = pbank.tile([128, L], f32, tag="pb")
                    for h in range(4):
                        p0 = 32 * h
                        nc.tensor.matmul(
                            b_p[p0:p0 + 17, :],
                            ones_s[p0:p0 + 1, :],
                            rec[p0:p0 + 1, :],
                            start=True, stop=True, tile_position=(p0, p0))
                    bsb = sb.tile([128, L], bf16, tag="bsb")
                    nc.scalar.copy(bsb, b_p)
                    onrm = sb.tile([128, L], bf16, tag="onrm")
                    nc.vector.tensor_mul(onrm, ot_p, bsb)

                    # --- projection accumulate
                    pw_s = pw_a_s if half == 0 else pw_b_s
                    nc.tensor.matmul(yt_p, pw_s, onrm,
                                     start=(half == 0), stop=(half == 1))

                # --- bias add (bf16 out), PE transpose back, scatter out
                yt16 = sb.tile([128, L], bf16, tag="yt16")
                with nc.allow_low_precision(reason="bf16 output"):
                    nc.vector.tensor_scalar_add(yt16, yt_p, pb_s)
                yn0_p = pbank.tile([98, 128], bf16, tag="pb")
                yn1_p = pbank.tile([98, 128], bf16, tag="pb")
                nc.tensor.transpose(yn0_p, yt16[:, 0:98], id_s)
                nc.tensor.transpose(yn1_p, yt16[:, 98:196], id_s)
                # --- int8 per-token quantization: q = y * (127/absmax),
                # f32 scale bit-packed into bytes 128:132 of each row
                for wt0, src in ((0, yn0_p), (2, yn1_p)):
                    m = sb.tile([98, 1], f32, tag="qm")
                    nc.vector.tensor_reduce(
                        m, src, mybir.AxisListType.X, mybir.AluOpType.max,
                        apply_absolute_value=True)
                    sc = sb.tile([98, 1], f32, tag="qsc")   # absmax/127
                    nc.vector.tensor_scalar(
                        sc, m, 1.0 / 127.0, 1e-30,
                        op0=mybir.AluOpType.mult, op1=mybir.AluOpType.max)
                    r = sb.tile([98, 1], f32, tag="qr")
                    nc.vector.reciprocal(r, sc)
                    q = sb.tile([98, DQ], i8, tag="q")
                    with nc.allow_low_precision(reason="int8 quantize"):
                        nc.vector.tensor_scalar_mul(q[:, 0:D], src, r)
                    nc.vector.tensor_copy(
                        q[:, D:DQ], sc.bitcast(i8))
                    for k in range(2):
                        nc.sync.dma_start(
                            out=yout[wt0 + k, hs:hs + 7, ws:ws + 7, :],
                            in_=q[49 * k:49 * k + 49, :])

    _split_mm_waits(nc, mybir)
    return nc


def _split_mm_waits(nc, mybir):
    """Walrus allows only one sync-wait on a Matmult: move extra waits onto
    PE NoOps inserted just before the matmul (same engine stream, absolute
    sem-ge waits, so waiting earlier is equivalent)."""
    for fn in nc.m.functions:
        for bb in fn.blocks:
            il = bb.instructions
            i = 0
            while i < len(il):
                inst = il[i]
                si = getattr(inst, "sync_info", None)
                if (not isinstance(inst, mybir.InstNoOp) and si is not None
                        and si.on_wait and len(si.on_wait) > 1):
                    waits = list(si.on_wait)
                    for wsel in waits[:-1]:
                        nop = mybir.InstNoOp(
                            name=nc.get_next_instruction_name(),
                            sync_info=mybir.SyncInfo(
                                on_wait=[wsel], on_update=[]),
                            bass_nofuse=True,
                            engine=inst.engine,
                        )
                        il.insert(i, nop)
                        i += 1
                    inst.sync_info = mybir.SyncInfo(
                        on_wait=[waits[-1]], on_update=list(si.on_update))
                i += 1


def _get_runner():
    if "runner" in _cache:
        return _cache["runner"]

    import jax
    from jax.sharding import Mesh, PartitionSpec
    from jax.experimental.shard_map import shard_map
    from concourse import mybir
    from concourse.bass2jax import (
        _bass_exec_p, install_neuronx_cc_hook, partition_id_tensor)

    install_neuronx_cc_hook()
    nc = _build_program()

    partition_name = (nc.partition_id_tensor.name
                      if nc.partition_id_tensor else None)
    in_names, out_names, out_avals = [], [], []
    for alloc in nc.m.functions[0].allocations:
        if not isinstance(alloc, mybir.MemoryLocationSet):
            continue
        name = alloc.memorylocations[0].name
        if alloc.kind == "ExternalInput":
            if name != partition_name:
                in_names.append(name)
        elif alloc.kind == "ExternalOutput":
            out_names.append(name)
            out_avals.append(jax.core.ShapedArray(
                tuple(alloc.tensor_shape), mybir.dt.np(alloc.dtype)))
    assert in_names == IN_NAMES, in_names
    assert out_names == OUT_NAMES, out_names
    in_names_all = list(in_names)
    if partition_name is not None:
        in_names_all.append(partition_name)

    def _body(*args):
        operands = list(args)
        if partition_name is not None:
            operands.append(partition_id_tensor())
        outs = _bass_exec_p.bind(
            *operands,
            out_avals=tuple(out_avals),
            in_names=tuple(in_names_all),
            out_names=tuple(out_names),
            lowering_input_output_aliases=(),
            sim_require_finite=True,
            sim_require_nnan=True,
            nc=nc,
        )
        return tuple(outs)

    devices = jax.devices()[:NCORES]
    mesh = Mesh(np.asarray(devices), ("core",))
    sharded = jax.jit(
        shard_map(
            _body, mesh=mesh,
            in_specs=(PartitionSpec("core"),) * len(in_names),
            out_specs=(PartitionSpec("core"),) * len(out_names),
            check_rep=False,
        ),
        keep_unused=True,
    )
    _cache["mesh"] = mesh
    _cache["runner"] = sharded
    return sharded


def _weights_glob(qkv_w, proj_w, proj_b):
    """Device-resident replicated weight arrays, re-uploaded only when the
    weight bytes change."""
    import hashlib
    import jax
    from jax.sharding import NamedSharding, PartitionSpec

    dig = hashlib.blake2b(qkv_w.tobytes(), digest_size=16)
    dig.update(proj_w.tobytes())
    dig.update(proj_b.tobytes())
    key = dig.hexdigest()
    ent = _cache.get("wglob")
    if ent is not None and ent[0] == key:
        return ent[1]
    wd = _prep_weights(qkv_w, proj_w, proj_b)
    sh = NamedSharding(_cache["mesh"], PartitionSpec("core"))
    dws = [jax.device_put(np.concatenate([wd[n]] * NCORES, axis=0), sh)
           for n in IN_NAMES[1:]]
    jax.block_until_ready(dws)
    _cache["wglob"] = (key, dws)
    return dws


def _prep_weights(qkv_w, proj_w, proj_b):
    Wq = qkv_w[0:128] * (HD ** -0.5)
    Wk = qkv_w[128:256]
    Wv = qkv_w[256:384]

    def head_pad_T(Wm):
        # out[di, 32h+j] = Wm[16h+j, di] for 4 heads, rest zero
        out_a = np.zeros((128, 128), np.float32)
        out_b = np.zeros((128, 128), np.float32)
        for h in range(4):
            out_a[:, 32 * h:32 * h + 16] = Wm[16 * h:16 * h + 16].T
            out_b[:, 32 * h:32 * h + 16] = Wm[16 * (h + 4):16 * (h + 4) + 16].T
        return out_a.astype(BF16), out_b.astype(BF16)

    wq_a, wq_b = head_pad_T(Wq)
    wk_a, wk_b = head_pad_T(Wk)
    wv = Wv.T.astype(BF16)

    # proj lhsT: row 32h+1+j of O^T layout corresponds to di = 16h+j
    pw_a = np.zeros((128, 128), np.float32)
    pw_b = np.zeros((128, 128), np.float32)
    for h in range(4):
        pw_a[32 * h + 1:32 * h + 17, :] = proj_w[:, 16 * h:16 * h + 16].T
        pw_b[32 * h + 1:32 * h + 17, :] = \
            proj_w[:, 16 * (h + 4):16 * (h + 4) + 16].T
    pw_a = pw_a.astype(BF16)
    pw_b = pw_b.astype(BF16)
    pb = proj_b.reshape(128, 1).astype(np.float32)
    ident = np.eye(128, dtype=np.float32).astype(BF16)
    return dict(wq_a=wq_a, wq_b=wq_b, wk_a=wk_a, wk_b=wk_b, wv=wv,
                pw_a=pw_a, pw_b=pw_b, pb=pb, ident=ident)


# wrap-split slices for the fused cast+roll passes
def _roll_chunks(size, shift):
    """dst[i] = src[(i + shift) % size] as a list of (dst_slice, src_slice)."""
    shift %= size
    if shift == 0:
        return [((0, size), (0, size))]
    return [((0, size - shift), (shift, size)),
            ((size - shift, size), (0, shift))]


def _pool():
    if "pool" not in _cache:
        from concurrent.futures import ThreadPoolExecutor
        _cache["pool"] = ThreadPoolExecutor(max_workers=16)
    return _cache["pool"]


def kernel(x, qkv_w, proj_w, proj_b):
    import jax
    from jax.sharding import NamedSharding, PartitionSpec

    x = np.asarray(x, np.float32)
    qkv_w = np.asarray(qkv_w, np.float32)
    proj_w = np.asarray(proj_w, np.float32)
    proj_b = np.asarray(proj_b, np.float32)

    runner = _get_runner()
    mesh = _cache["mesh"]
    devices = list(mesh.devices)
    wglob = _weights_glob(qkv_w, proj_w, proj_b)

    # prep each core's slab (fused cast+roll) and start its upload
    # immediately, overlapping the remaining host prep under the tunnel
    hw = [(dh, sh, dw, sw)
          for dh, sh in _roll_chunks(H, 4)
          for dw, sw in _roll_chunks(W, 4)]
    x5 = x.reshape(N, T, H, W, D)
    if "slabs" not in _cache:
        _cache["slabs"] = [np.empty((WT, H, W, D), BF16)
                           for _ in range(NCORES)]
    parts = []
    for c in range(NCORES):
        n_, tblk = c // 4, c % 4
        slab = _cache["slabs"][c]
        for wt in range(WT):
            st = (4 * tblk + wt + 2) % T
            for dh, sh, dw, sw in hw:
                slab[wt, dh[0]:dh[1], dw[0]:dw[1]] = \
                    x5[n_, st, sh[0]:sh[1], sw[0]:sw[1]]
        parts.append(jax.device_put(slab, devices[c]))
    xg = jax.make_array_from_single_device_arrays(
        (NCORES * WT, H, W, D),
        NamedSharding(mesh, PartitionSpec("core")), parts)

    (yg,) = runner(xg, *wglob)        # [32,56,56,132] int8 (data + scale)

    shards = list(yg.addressable_shards)
    for s in shards:
        s.data.copy_to_host_async()

    # per-shard: fetch + dequant + reverse roll (T by +2, H/W by +3)
    y5 = np.empty((N, T, H, W, D), np.float32)
    h_chunks = _roll_chunks(H, -3)   # dst[i] = src[(i-3) % 56]

    def do_shard(s):
        c = s.index[0].start // WT
        n_, tblk = c // 4, c % 4
        arr = np.asarray(s.data)                   # [4,56,56,132] int8
        sc = np.ascontiguousarray(arr[:, :, :, D:DQ]).view('<f4')[..., 0]
        # this shard holds rolled T = 4*tblk + wt -> final T = (+2) % 16
        for wt in range(WT):
            tf = (4 * tblk + wt + 2) % T
            for dh, sh in h_chunks:
                for dw, sw in h_chunks:
                    blk = arr[wt, sh[0]:sh[1], sw[0]:sw[1], 0:D]
                    y5[n_, tf, dh[0]:dh[1], dw[0]:dw[1]] = \
                        blk * sc[wt, sh[0]:sh[1], sw[0]:sw[1], None]
    list(_pool().map(do_shard, shards))
    return y5.reshape(N, T, S, D)
